# revision 23
# baseline (speedup 1.0000x reference)
"""AttentiveStatPool Trainium2 kernel.

Full inputs -> full output; shards batch B=32 across 8 NeuronCores
(4 utterances per core), runs one SPMD Bass/Tile kernel, gathers.

Math (per utterance, per channel c):
  mean/std over T -> glob = [x; mean; std] -> h = relu(W1 @ glob + b1)
  logits = W2 @ h (+ b2, which cancels in the softmax over T and is dropped)
  w = softmax_T(logits); out = [sum_t x*w, sqrt(clip(sum_t x^2*w - mean_w^2))]

Implementation notes:
  - e = exp(logits) unnormalized; S1 = sum x*e, S2 = sum x^2*e, s = sum e
    computed with fused DVE scalar_tensor_tensor accumulate ops; the
    normalization (1/s) is applied to the tiny [128, 12] results.
  - x is cast to bf16 by an ACT Copy-with-accumulate pass that also yields
    sum(x); sum(x^2) comes from ACT Square-accum / DVE STT (split to
    balance the two engines). Matmuls are bf16 (fp32 PSUM accumulate).
  - std = exp(0.5*ln(var)) so every ACT function (copy, square, relu, ln,
    exp) lives in one table set (no table-switch stalls).
"""

import numpy as np
import ml_dtypes
from contextlib import ExitStack

import concourse.bass as bass
import concourse.tile as tile
from concourse import mybir
from concourse.bass_utils import run_bass_kernel_spmd

B, C, T, BOT = 32, 1536, 2000, 128
NCORES = 8
BS = B // NCORES          # utterances per core
NCH = C // 128            # channel chunks
EPS = 1e-4
F32 = mybir.dt.float32
BF16 = mybir.dt.bfloat16
MULT = mybir.AluOpType.mult
AF = mybir.ActivationFunctionType

_counter = [0]


def _split_excess_waits(nc, cap_regular=1, cap_es=2):
    """Walrus allows 1 sem-wait per regular instruction (2 on
    EventSemaphore). Hoist excess waits onto EventSemaphore insts."""
    for f in nc.m.functions:
        for blk in f.blocks:
            insts = blk.instructions
            out = []
            for inst in insts:
                si = inst.sync_info
                cap = (
                    cap_es
                    if isinstance(inst, mybir.InstEventSemaphore)
                    else cap_regular
                )
                if si is not None and len(si.on_wait) > cap:
                    waits = list(si.on_wait)
                    keep, extra = waits[:cap], waits[cap:]
                    for i in range(0, len(extra), 2):
                        _counter[0] += 1
                        es = mybir.InstEventSemaphore(
                            name=f"waitsplit_{_counter[0]}", engine=inst.engine
                        )
                        es.sync_info = mybir.SyncInfo(
                            on_wait=extra[i : i + 2], on_update=[]
                        )
                        out.append(es)
                    inst.sync_info = mybir.SyncInfo(
                        on_wait=keep, on_update=list(si.on_update)
                    )
                out.append(inst)
            if len(out) != len(insts):
                insts.clear()
                insts.extend(out)


def _build(ctx, tc):
    nc = tc.nc
    x_in = nc.dram_tensor("x", [BS, C, T], F32, kind="ExternalInput").ap()
    w1xt_in = nc.dram_tensor("w1xt", [C, BOT], BF16, kind="ExternalInput").ap()
    w1mt_in = nc.dram_tensor("w1mt", [C, BOT], BF16, kind="ExternalInput").ap()
    w1st_in = nc.dram_tensor("w1st", [C, BOT], BF16, kind="ExternalInput").ap()
    w2t_in = nc.dram_tensor("w2t", [BOT, C], BF16, kind="ExternalInput").ap()
    b1_in = nc.dram_tensor("b1", [BOT, 1], F32, kind="ExternalInput").ap()
    ident_in = nc.dram_tensor("ident", [128, 128], F32, kind="ExternalInput").ap()
    out_dram = nc.dram_tensor("out", [BS, 2 * C], F32, kind="ExternalOutput").ap()

    wpool = ctx.enter_context(tc.tile_pool(name="weights", bufs=1))
    xfpool = ctx.enter_context(tc.tile_pool(name="xf", bufs=4))
    xbpool = ctx.enter_context(tc.tile_pool(name="xb", bufs=24))
    epool = ctx.enter_context(tc.tile_pool(name="e", bufs=2))
    upool = ctx.enter_context(tc.tile_pool(name="u", bufs=2))
    hpool = ctx.enter_context(tc.tile_pool(name="h", bufs=2))
    spool = ctx.enter_context(tc.tile_pool(name="stats", bufs=1))
    tpool = ctx.enter_context(tc.tile_pool(name="tmp", bufs=1))
    hpsum = ctx.enter_context(tc.tile_pool(name="hpsum", bufs=1, space="PSUM"))
    lgpsum = ctx.enter_context(tc.tile_pool(name="lgpsum", bufs=1, space="PSUM"))

    # --- weights to SBUF ---
    w1xt = wpool.tile([128, NCH * BOT], BF16, tag="w1xt")
    w1mt = wpool.tile([128, NCH * BOT], BF16, tag="w1mt")
    w1st = wpool.tile([128, NCH * BOT], BF16, tag="w1st")
    w2t = wpool.tile([BOT, C], BF16, tag="w2t")
    b1sb = wpool.tile([BOT, 1], F32, tag="b1sb")
    ident = wpool.tile([128, 128], F32, tag="ident")
    # weight loads: single coalesced DMAs on the (otherwise idle) gpsimd queue
    for wt, win in ((w1xt, w1xt_in), (w1mt, w1mt_in), (w1st, w1st_in)):
        nc.gpsimd.dma_start(
            wt[:].rearrange("c (j o) -> c j o", o=BOT),
            win.rearrange("(j c) o -> c j o", c=128),
        )
    nc.gpsimd.dma_start(w2t[:], w2t_in[:])
    nc.gpsimd.dma_start(b1sb[:], b1_in[:])
    nc.gpsimd.dma_start(ident[:], ident_in[:])

    # --- persistent accumulators ([128, col]) ---
    sx = spool.tile([128, BS * NCH], F32, tag="sx")        # sum x
    sxx = spool.tile([128, BS * NCH], F32, tag="sxx")      # sum x^2
    sE = spool.tile([128, BS * NCH], F32, tag="sE")        # sum e
    S1 = spool.tile([128, BS * NCH], F32, tag="S1")        # sum x*e
    S2 = spool.tile([128, BS * NCH], F32, tag="S2")        # sum x^2*e
    scr_act = spool.tile([128, T], BF16, tag="scr_act")    # ACT dump
    scr_dve = spool.tile([128, T], BF16, tag="scr_dve")    # DVE dump

    NTOT = BS * NCH
    # N-subtile boundaries (bank-aligned, <=512)
    NS = [(0, 512), (512, 512), (1024, 512), (1536, 464)]

    hpss = {}
    hsbs = {}
    xbss = {}
    cbs = {}

    def emit_A_j(b, j):
        if j == 0:
            # h psum is [128, 2048] (exactly 4 banks); the last column
            # doubles as the c_b accumulator (disjoint from h [0:2000]).
            hpss[b] = hpsum.tile([128, 2048], F32, tag="hps", name="hps")
            xbss[b] = []
        hps = hpss[b]
        xbs = xbss[b]
        col = b * NCH + j
        xf = xfpool.tile([128, T], F32, tag="xf")
        nc.sync.dma_start(xf[:], x_in[b, bass.ts(j, 128), :])
        xb = xbpool.tile([128, T], BF16, tag="xb")
        xbs.append(xb)
        # cast + sum(x) on ACT
        nc.scalar.activation(
            xb[:], xf[:], AF.Copy, accum_out=sx[:, col : col + 1]
        )
        # sum(x^2): alternate ACT / DVE to balance engines
        if j % 2 == 0:
            nc.scalar.activation(
                scr_act[:], xf[:], AF.Square,
                accum_out=sxx[:, col : col + 1],
            )
        else:
            nc.vector.scalar_tensor_tensor(
                scr_dve[:], xb[:], 1.0, xb[:],
                op0=MULT, op1=MULT,
                accum_out=sxx[:, col : col + 1],
            )
        # stage B: h += W1x_j.T-chunk @ x_j
        for (n0, nn) in NS:
            nc.tensor.matmul(
                hps[:, n0 : n0 + nn],
                w1xt[:, bass.ts(j, BOT)],
                xb[:, n0 : n0 + nn],
                start=(j == 0),
                stop=(j == NCH - 1),
            )

    def emit_A(b):
        for j in range(NCH):
            emit_A_j(b, j)

    def emit_B(b):
        hps = hpss[b]
        # --- stats -> mean, std (bf16 for the matvec) ---
        bsl = slice(b * NCH, (b + 1) * NCH)
        mean_b = tpool.tile([128, NCH], BF16, tag=f"mean{b}")
        std_b = tpool.tile([128, NCH], BF16, tag=f"std{b}")
        t1 = tpool.tile([128, NCH], F32, tag=f"t1_{b}")
        t2 = tpool.tile([128, NCH], F32, tag=f"t2_{b}")
        t3 = tpool.tile([128, NCH], F32, tag=f"t3_{b}")
        t4 = tpool.tile([128, NCH], F32, tag=f"t4_{b}")
        nc.vector.tensor_scalar(mean_b[:], sx[:, bsl], 1.0 / T, None, op0=MULT)
        # var = sxx/(T-1) - sx^2/(T*(T-1))
        nc.vector.tensor_scalar(t1[:], sxx[:, bsl], 1.0 / (T - 1), None, op0=MULT)
        nc.vector.scalar_tensor_tensor(
            t2[:], sx[:, bsl], -1.0 / (T * (T - 1.0)), sx[:, bsl],
            op0=MULT, op1=MULT,
        )
        nc.vector.tensor_add(t3[:], t1[:], t2[:])
        nc.vector.tensor_scalar_max(t4[:], t3[:], EPS)
        lnv = tpool.tile([128, NCH], F32, tag=f"lnv{b}")
        nc.scalar.activation(lnv[:], t4[:], AF.Ln)
        nc.scalar.activation(std_b[:], lnv[:], AF.Exp, scale=0.5)

        # --- c_b = W1m @ mean + W1s @ std  (24 N=1 matmuls) ---
        cbp = hps[:, 2047:2048]
        for j in range(NCH):
            nc.tensor.matmul(
                cbp, w1mt[:, bass.ts(j, BOT)], mean_b[:, j : j + 1],
                start=(j == 0), stop=False,
            )
        for j in range(NCH):
            nc.tensor.matmul(
                cbp, w1st[:, bass.ts(j, BOT)], std_b[:, j : j + 1],
                start=False, stop=(j == NCH - 1),
            )
        cb = tpool.tile([128, 1], F32, tag=f"cb{b}")
        nc.vector.tensor_add(cb[:], cbp, b1sb[:])
        cbs[b] = cb

    def emit_B2(b):
        # --- h = relu(hpsum + c_b) -> bf16 ---
        hsb = hpool.tile([BOT, T], BF16, tag="hsb")
        hsbs[b] = hsb
        nc.scalar.activation(hsb[:], hpss[b][:, 0:T], AF.Relu, bias=cbs[b][:])

    def emit_C_j(b, j, last_b=False):
        hsb = hsbs[b]
        xbs = xbss[b]
        col = b * NCH + j
        wsl = bass.ts(j, BOT)  # chunk of w2t columns (c-block)
        # last b: the h-psum slot is free, ping-pong logits across both
        # psum pools so PE(j+1) overlaps exp(j)
        if last_b and j % 2 == 1:
            lg = hpsum.tile([128, 2048], F32, tag="hps")
        else:
            lg = lgpsum.tile([128, 2048], F32, tag="lg")
        for (n0, nn) in NS:
            nc.tensor.matmul(
                lg[:, n0 : n0 + nn], w2t[:, wsl], hsb[:, n0 : n0 + nn],
                start=True, stop=True,
            )
        e = epool.tile([128, T], BF16, tag="e")
        nc.scalar.activation(
            e[:], lg[:, 0:T], AF.Exp, accum_out=sE[:, col : col + 1]
        )
        u = upool.tile([128, T], BF16, tag="u")
        nc.vector.scalar_tensor_tensor(
            u[:], xbs[j][:], 1.0, e[:],
            op0=MULT, op1=MULT, accum_out=S1[:, col : col + 1],
        )
        nc.vector.scalar_tensor_tensor(
            scr_dve[:], xbs[j][:], 1.0, u[:],
            op0=MULT, op1=MULT, accum_out=S2[:, col : col + 1],
        )

    def emit_CA(b, next_b):
        # interleave C(b, j) with A(b+1, j): ACT alternates exp / copy /
        # square so the single-buffered logits psum never starves it
        for j in range(NCH):
            emit_C_j(b, j, last_b=(next_b is None))
            if next_b is not None:
                emit_A_j(next_b, j)

    rs = spool.tile([128, NTOT], F32, tag="rs")
    wmean = spool.tile([128, NTOT], F32, tag="wmean")
    e2w = spool.tile([128, NTOT], F32, tag="e2w")
    nm2 = spool.tile([128, NTOT], F32, tag="nm2")
    varw = spool.tile([128, NTOT], F32, tag="varw")
    varc = spool.tile([128, NTOT], F32, tag="varc")
    lnw = spool.tile([128, NTOT], F32, tag="lnw")
    wsd = spool.tile([128, NTOT], F32, tag="wsd")

    def emit_F(b):
        sl = slice(b * NCH, (b + 1) * NCH)
        nc.vector.reciprocal(rs[:, sl], sE[:, sl])
        nc.vector.tensor_mul(wmean[:, sl], S1[:, sl], rs[:, sl])
        nc.vector.tensor_mul(e2w[:, sl], S2[:, sl], rs[:, sl])
        nc.vector.scalar_tensor_tensor(
            nm2[:, sl], wmean[:, sl], -1.0, wmean[:, sl], op0=MULT, op1=MULT
        )
        nc.vector.tensor_add(varw[:, sl], e2w[:, sl], nm2[:, sl])
        nc.vector.tensor_scalar_max(varc[:, sl], varw[:, sl], EPS)
        nc.scalar.activation(lnw[:, sl], varc[:, sl], AF.Ln)
        nc.scalar.activation(wsd[:, sl], lnw[:, sl], AF.Exp, scale=0.5)

    # software-pipelined emission: C(b) and A(b+1) interleave per chunk;
    # stats/bias chains (B) land immediately after their inputs exist.
    emit_A(0)
    emit_B(0)
    emit_B2(0)
    emit_CA(0, next_b=1)
    emit_B(1)
    emit_B2(1)
    emit_F(0)
    emit_CA(1, next_b=2)
    emit_B(2)
    emit_B2(2)
    emit_F(1)
    emit_CA(2, next_b=3)
    emit_B(3)
    emit_B2(3)
    emit_F(2)
    emit_CA(3, next_b=None)
    emit_F(3)
    # transpose [128, 48] -> [48, 128] on PE, then 2 contiguous stores
    wmT = lgpsum.tile([NTOT, 128], F32, tag="lg")
    nc.tensor.transpose(wmT[:], wmean[:], ident[:])
    wsT = lgpsum.tile([NTOT, 128], F32, tag="lg")
    nc.tensor.transpose(wsT[:], wsd[:], ident[:])
    wmTs = spool.tile([NTOT, 128], F32, tag="wmTs")
    wsTs = spool.tile([NTOT, 128], F32, tag="wsTs")
    nc.vector.tensor_copy(wmTs[:], wmT[:])
    nc.vector.tensor_copy(wsTs[:], wsT[:])
    for b in range(BS):
        nc.sync.dma_start(
            out_dram[b, 0:C].rearrange("(j p) -> j p", p=128),
            wmTs[b * NCH : (b + 1) * NCH, :],
        )
        nc.sync.dma_start(
            out_dram[b, C : 2 * C].rearrange("(j p) -> j p", p=128),
            wsTs[b * NCH : (b + 1) * NCH, :],
        )


_NC_CACHE = {}


def _get_nc():
    if "nc" not in _NC_CACHE:
        nc = bass.Bass("TRN2", target_bir_lowering=False, debug=False)
        with tile.TileContext(nc) as tc:
            with ExitStack() as ctx:
                _build(ctx, tc)
        _split_excess_waits(nc)
        _NC_CACHE["nc"] = nc
    return _NC_CACHE["nc"]


def kernel(x, W1, b1, W2, b2, _trace=False, _trace_kwargs=None):
    x = np.asarray(x, dtype=np.float32)
    W1 = np.asarray(W1, dtype=np.float32)
    b1 = np.asarray(b1, dtype=np.float32)
    W2 = np.asarray(W2, dtype=np.float32)
    b2 = np.asarray(b2, dtype=np.float32)  # cancels in softmax; unused

    bf = ml_dtypes.bfloat16
    w1xt = np.ascontiguousarray(W1[:, 0:C].T).astype(bf)          # [C, BOT]
    w1mt = np.ascontiguousarray(W1[:, C : 2 * C].T).astype(bf)    # [C, BOT]
    w1st = np.ascontiguousarray(W1[:, 2 * C : 3 * C].T).astype(bf)
    w2t = np.ascontiguousarray(W2.T).astype(bf)                   # [BOT, C]
    b1c = np.ascontiguousarray(b1.reshape(BOT, 1))
    ident = np.eye(128, dtype=np.float32)

    nc = _get_nc()
    in_maps = [
        {
            "x": np.ascontiguousarray(x[i * BS : (i + 1) * BS]),
            "w1xt": w1xt,
            "w1mt": w1mt,
            "w1st": w1st,
            "w2t": w2t,
            "b1": b1c,
            "ident": ident,
        }
        for i in range(NCORES)
    ]
    res = run_bass_kernel_spmd(
        nc,
        in_maps,
        list(range(NCORES)),
        trace=_trace,
        **(_trace_kwargs or {}),
    )
    out = np.concatenate([res.results[i]["out"] for i in range(NCORES)], axis=0)
    if _trace:
        kernel.last_results = res
    return out
